# revision 39
# baseline (speedup 1.0000x reference)
import sys

sys.path.insert(0, "/opt/trn_rl_repo")
import numpy as np

N1, N2, D = 8192, 8192, 256
NCORES = 8
QPC = N1 // NCORES  # queries per core (1024)
RT = QPC // 128  # row tiles per core (8)
GW = 2048  # colgroup width (4 psum banks)
NCG = 8  # colgroups (4 per side)

_OP_NAME = "TT_ADD_MAX_DIAG_ANT"


def _register_custom_op():
    """Fused DVE op: body = (idx==c0 ? -FLT_MAX : in0 + in1), accum = max.

    in0 = psum dot tile, in1 = broadcast -0.5||y||^2 row, c0 = per-partition
    self-column index (or -1 to disable masking). Registered at runtime via
    the documented dve_ops extension point (append to OPS)."""
    import concourse.dve_ops as dve_ops

    for op in dve_ops.OPS:
        if op.name == _OP_NAME:
            return op
    from concourse.dve_spec import (
        C0,
        Idx,
        MaxNeg,
        Spec,
        Src0,
        Src1,
        eq,
        lower,
        maxx,
        select,
        _has_src1,
    )
    from concourse.dve_uop import DveOpSpec

    def _ref(in0, in1, c0, c1, c2):
        P = in0.shape[0]
        x = in0.astype(np.float32).reshape(P, -1)
        y = np.broadcast_to(np.asarray(in1, np.float32).reshape(P, -1), x.shape)
        n = x.shape[1]
        idx = np.broadcast_to(np.arange(n, dtype=np.float32), (P, n))
        c0b = np.broadcast_to(np.asarray(c0, np.float32).reshape(-1, 1), (P, 1))
        fmin = np.float32(np.finfo(np.float32).min)
        body = np.where(idx == c0b, fmin, x + y).astype(np.float32)
        acc = np.maximum(np.max(body, axis=-1, keepdims=True), fmin)
        return body, acc

    spec = Spec(
        body=select(eq(Idx, C0), MaxNeg, Src0 + Src1),
        accum=maxx,
        reference=_ref,
    )
    row = dve_ops._CUSTOM_DVE_ROW_BASE + len(dve_ops.OPS)
    shas = {}
    for ver in ("v3", "v4"):
        try:
            u = lower(spec, ver=ver)
            shas[ver] = DveOpSpec(
                name=_OP_NAME, opcode=row, uops=u, rd1_en=_has_src1(spec)
            ).sha(ver)
        except ValueError:
            pass
    op = dve_ops.DveOp(_OP_NAME, spec, subdim=False, uops_sha=shas)
    dve_ops.OPS.append(op)
    dve_ops._SUB_OPCODE_FOR_NAME[_OP_NAME] = row
    dve_ops.CUSTOM_DVE_SPECS[_OP_NAME] = spec
    return op


def _build_nc():
    import concourse.bass as bass
    import concourse.tile as tile
    from concourse import mybir

    ttop = _register_custom_op()

    f32 = mybir.dt.float32
    bf16 = mybir.dt.bfloat16
    fp16 = mybir.dt.float16

    nc = bass.Bass()
    dbd = [
        nc.dram_tensor(f"db{c}", [128, 2, GW], bf16, kind="ExternalInput")
        for c in range(NCG)
    ]
    nbcd = [
        nc.dram_tensor(f"nbc{c}", [128, GW], fp16, kind="ExternalInput")
        for c in range(NCG)
    ]
    diagwd = nc.dram_tensor("diagw", [128, RT], f32, kind="ExternalInput")
    o = nc.dram_tensor("o", [128, RT, NCG], f32, kind="ExternalOutput")

    with tile.TileContext(nc) as tc:
        with (
            tc.tile_pool(name="sb", bufs=1) as sb,
            tc.tile_pool(name="pp", bufs=1) as pp,
            tc.tile_pool(name="ps", bufs=2, space="PSUM") as ps,
        ):
            # DMA plan: small/critical first; db/nbc chunks land in
            # consumption order, spread over four DGE queues.
            tdw = sb.tile([128, RT], f32, tag="dw")
            nc.gpsimd.dma_start(out=tdw, in_=diagwd[:])
            tnbc = [
                sb.tile([128, GW], fp16, name=f"nbc{c}", tag=f"nbc{c}")
                for c in range(NCG)
            ]
            nc.gpsimd.dma_start(out=tnbc[0], in_=nbcd[0][:])
            qs = [nc.sync, nc.scalar]
            tdb = [
                sb.tile([128, 2, GW], bf16, name=f"db{c}", tag=f"db{c}")
                for c in range(NCG)
            ]
            for c in range(NCG):
                qs[c % 2].dma_start(out=tdb[c], in_=dbd[c][:])
            for c in range(1, NCG):
                nc.gpsimd.dma_start(out=tnbc[c], in_=nbcd[c][:])

            # wait absorbers: DVE observes the startup-critical DMAs once
            dum = sb.tile([128, 2], f32, tag="dum")
            nc.vector.tensor_copy(out=dum[:, 0:1], in_=tdw[:, 0:1])
            nc.vector.tensor_copy(out=dum[:, 1:2], in_=tnbc[0][:, 0:1])
            dumo = sb.tile([128, 1], f32, tag="dumo")

            parts = [
                pp.tile([128, NCG], f32, name=f"part{m}", tag=f"part{m}")
                for m in range(RT)
            ]
            for cg in range(NCG):
                for m in range(RT):
                    lhs = [tdb[0][:, k, m * 128 : (m + 1) * 128] for k in (0, 1)]
                    pst = ps.tile([128, GW], f32, tag="pst")
                    for k in (0, 1):
                        for i in (0, 1, 2, 3):
                            nc.tensor.matmul(
                                out=pst[:, i * 512 : (i + 1) * 512],
                                lhsT=lhs[k],
                                rhs=tdb[cg][:, k, i * 512 : (i + 1) * 512],
                                start=(k == 0),
                                stop=(k == 1),
                            )
                    nc.vector._custom_dve(
                        ttop,
                        out=dumo.broadcast_to(pst.shape),
                        in0=pst,
                        in1=tnbc[cg][:, :],
                        s0=tdw[:, m : m + 1] if cg == 0 else -1.0,
                        s1=0.0,
                        imm2=0.0,
                        accum_out=parts[m][:, cg : cg + 1],
                    )
            for m in range(RT):
                nc.sync.dma_start(out=o[:, m, :], in_=parts[m])

    from concourse.bass import _bass_rust
    from concourse.library_overlay import lower_extended_insts

    lower_extended_insts(nc)  # populate .instr for InstISA subclasses
    _bass_rust.move_matmul_waits_to_ldweights(nc.m)
    _bass_rust.generate_event_semaphores(nc)
    return nc


def _prep_core(s1, s2T_bf, c2_half, c):
    import ml_dtypes

    bf = ml_dtypes.bfloat16
    s1p = np.roll(s1, -c * QPC, axis=0)
    s1pT_bf = np.ascontiguousarray(s1p.T).astype(bf)
    dbx = np.empty((128, 2, 2 * N1), dtype=bf)
    for k in (0, 1):
        dbx[:, k, 0:N1] = s1pT_bf[k * 128 : (k + 1) * 128]
        dbx[:, k, N1 : 2 * N1] = s2T_bf[k * 128 : (k + 1) * 128]
    dbd = {
        f"db{c_}": np.ascontiguousarray(dbx[:, :, c_ * GW : (c_ + 1) * GW])
        for c_ in range(NCG)
    }
    # -0.5||y||^2 broadcast rows: rolled s1 side then s2 side, fp16
    c1_64 = -0.5 * np.square(s1p.astype(np.float64)).sum(1)
    crow_half = np.concatenate([c1_64.astype(np.float16), c2_half])
    nbc = np.ascontiguousarray(np.broadcast_to(crow_half[None, :], (128, 2 * N1)))
    for c_ in range(NCG):
        dbd[f"nbc{c_}"] = np.ascontiguousarray(nbc[:, c_ * GW : (c_ + 1) * GW])
    return dbd


def kernel(s1, s2, k):
    assert int(k) == 1
    from concourse.bass_utils import run_bass_kernel_spmd
    import ml_dtypes

    s1 = np.asarray(s1, dtype=np.float32)
    s2 = np.asarray(s2, dtype=np.float32)
    s2T_bf = np.ascontiguousarray(s2.T).astype(ml_dtypes.bfloat16)
    c2_half = (-0.5 * np.square(s2.astype(np.float64)).sum(1)).astype(np.float16)
    diagw = (
        np.arange(RT, dtype=np.float32)[None, :] * 128
        + np.arange(128, dtype=np.float32)[:, None]
    )
    diagw = np.ascontiguousarray(diagw)

    nc = _build_nc()
    in_maps = []
    for c in range(NCORES):
        dbd = _prep_core(s1, s2T_bf, c2_half, c)
        in_maps.append({**dbd, "diagw": diagw})
    import os

    res = run_bass_kernel_spmd(
        nc,
        in_maps,
        core_ids=list(range(NCORES)),
        trace=os.environ.get("KBENCH_TRACE") == "1",
    )
    kernel.last_results = res

    # host epilogue (float64): rho/nu from per-group maxes, then the estimator
    sq1 = np.square(s1.astype(np.float64)).sum(1)
    total = 0.0
    for c in range(NCORES):
        part = res.results[c]["o"].astype(np.float64)  # [128, RT, 8]
        maxA = part[:, :, 0:4].max(axis=2)  # [128, RT]
        maxB = part[:, :, 4:8].max(axis=2)
        idx = np.arange(RT)[None, :] * 128 + np.arange(128)[:, None]
        orig = (c * QPC + idx) % N1
        sqx = sq1[orig]
        rho_sq = sqx - 2.0 * maxA
        nu_sq = sqx - 2.0 * maxB
        rho_sq = np.maximum(rho_sq, 1e-20)
        nu_sq = np.maximum(nu_sq, 1e-20)
        total += 0.5 * (np.log(nu_sq) - np.log(rho_sq)).sum()
    base = np.log(N2 / (N1 - 1))
    return np.float32(base + (D / N1) * total)
